# revision 27
# baseline (speedup 1.0000x reference)
"""Causal MHA (GQA 16q/4kv, QK-RMSnorm, RoPE, tanh softcap 50) on 8 TRN2 cores.

Sharding: 8 shards = (batch b in {0,1}) x (kv-group g in {0..3}).
Each core handles one batch's one kv-head group: 4 Q heads + 1 KV head,
w_q/w_k/w_v column-sharded, w_o row-sharded; host sums the 4 partial
y outputs per batch.

v2 dataflow per core, fused + software-pipelined loop over 16 q-chunks m:
  PE order per iter: proj(m+1) -> scores(m) -> PV(m) -> y(m) -> q/k T(m+1)
  proj: fp16 x/wqkv matmuls; rms via ACT sqrt + DVE recip; rope -> fp16
  scores: qT stationary, kT streamed 512-wide; tanh (ACT); diag mask via
  copy_predicated; exp (ACT) with accum_out giving softmax denom free;
  p normalized in [q,k] layout (per-partition scalar), DMA-transposed.
  PV: v stationary [128,64], 4 heads' pT streamed 256-wide into two
  partition halves of one PSUM tile -> oT arrives y-ready (heads g,g+2
  stacked on partitions); wo rows host-permuted to match.
"""

import numpy as np

D_MODEL = 1024
SEQ = 2048
HD = 64
NQH = 4  # q heads per core
CAP = 50.0
EPS = 1e-5
THETA = 10000.0
P = 128
MC = SEQ // P  # 16 q-chunks
KT = D_MODEL // P  # 8 contraction chunks for projections
N_CORES = 8

_nc_cache = None


def _build_nc():
    import concourse.bass as bass
    import concourse.tile as tile
    from concourse import bacc, mybir
    from concourse.bass import ts
    from concourse.masks import make_identity

    F32 = mybir.dt.float32
    F16 = mybir.dt.float16
    AF = mybir.ActivationFunctionType
    ALU = mybir.AluOpType
    AX = mybir.AxisListType

    nc = bacc.Bacc("TRN2")
    xT_d = nc.declare_dram_parameter("xT", [D_MODEL, SEQ], F16, isOutput=False)
    wqkv_d = nc.declare_dram_parameter("wqkv", [D_MODEL, 384], F16, isOutput=False)
    wo_d = nc.declare_dram_parameter("wo", [256, D_MODEL], F16, isOutput=False)
    cs_d = nc.declare_dram_parameter("cs", [SEQ, 64], F32, isOutput=False)
    triu_d = nc.declare_dram_parameter("triu", [P, P], mybir.dt.uint8, isOutput=False)
    y_d = nc.declare_dram_parameter("y", [SEQ, D_MODEL], F32, isOutput=True)

    with tile.TileContext(nc) as tc:
        with (
            tc.tile_pool(name="singles", bufs=1) as singles,
            tc.tile_pool(name="xmp", bufs=3) as xmp,
            tc.tile_pool(name="ptmp", bufs=2) as ptmp,
            tc.tile_pool(name="small", bufs=4) as small,
            tc.tile_pool(name="qrp", bufs=2) as qrp,
            tc.tile_pool(name="tpool", bufs=3) as tpool,
            tc.tile_pool(name="ppool", bufs=2) as ppool,
            tc.tile_pool(name="ptp", bufs=2) as ptp,
            tc.tile_pool(name="opool", bufs=2) as opool,
            tc.tile_pool(name="ysb", bufs=2) as ysb,
            tc.tile_pool(name="psum_s", bufs=3, space="PSUM") as psum_s,
            tc.tile_pool(name="psum_pj", bufs=1, space="PSUM") as psum_pj,
            tc.tile_pool(name="psum_pv", bufs=2, space="PSUM") as psum_pv,
            tc.tile_pool(name="psum_y", bufs=2, space="PSUM") as psum_y,
        ):
            triu_sb = singles.tile([P, P], mybir.dt.uint8)
            nc.scalar.dma_start(triu_sb, triu_d[:, :])
            neg_sb = singles.tile([P, P], F32)
            nc.vector.memset(neg_sb, -100.0)
            wo_sb = singles.tile([P, 2, D_MODEL], F16)
            nc.scalar.dma_start(wo_sb, wo_d[:, :].rearrange("(o p) n -> p o n", p=P))
            wqkv_sb = singles.tile([P, KT, 384], F16)
            nc.scalar.dma_start(
                wqkv_sb, wqkv_d[:, :].rearrange("(o p) n -> p o n", p=P)
            )
            cs_sb = singles.tile([P, MC, 64], F32)
            nc.scalar.dma_start(cs_sb, cs_d[:, :].rearrange("(t p) n -> p t n", p=P))
            v_sb = singles.tile([P, MC, HD], F16)
            qT_sb = singles.tile([P, 2, SEQ], F16)
            kT_sb = singles.tile([P, SEQ], F16)

            xT_r = xT_d[:, :].rearrange("(o p) s -> p o s", p=P)

            def proj_front(m):
                """proj matmuls for chunk m (PE part only)."""
                xm = xmp.tile([P, KT, P], F16, tag="xm")
                nc.gpsimd.dma_start(xm, xT_r[:, :, ts(m, P)])
                pj = psum_pj.tile([P, 384], F32, tag="pj", name="pj")
                for kt in range(KT):
                    nc.tensor.matmul(
                        pj,
                        lhsT=xm[:, kt, :],
                        rhs=wqkv_sb[:, kt, :],
                        start=(kt == 0),
                        stop=(kt == KT - 1),
                    )
                return pj

            def proj_back(m, pj):
                """rms-norm + rope (DVE/ACT) + fp16 transposes for chunk m."""
                pjh = pj[:, 0:320].rearrange("p (h d) -> p h d", d=HD)
                sq = ptmp.tile([P, 5, HD], F32, tag="sq")
                nc.scalar.activation(sq, pjh, AF.Square)
                ssq = small.tile([P, 5], F32, tag="ssq")
                nc.vector.reduce_sum(ssq, sq, axis=AX.X)
                ms = small.tile([P, 5], F32, tag="ms")
                nc.vector.tensor_scalar(ms, ssq, 1.0 / HD, EPS, ALU.mult, ALU.add)
                sms = small.tile([P, 5], F32, tag="sms")
                nc.scalar.sqrt(sms, ms)
                rr = small.tile([P, 5], F32, tag="rr")
                nc.vector.reciprocal(rr, sms)
                qh = ptmp.tile([P, 5, HD], F32, tag="qh")
                nc.vector.tensor_mul(qh, pjh, rr[:, :, None].to_broadcast((P, 5, HD)))
                # v (unnormalized, no rope): cols 320:384
                nc.vector.tensor_copy(v_sb[:, m, :], pj[:, 320:384])
                # rope on the 5 q/k heads, output fp16
                cosb = cs_sb[:, m, None, 0:32].to_broadcast((P, 5, 32))
                sinb = cs_sb[:, m, None, 32:64].to_broadcast((P, 5, 32))
                q1 = qh[:, :, 0:32]
                q2 = qh[:, :, 32:64]
                qr = qrp.tile([P, 6, HD], F16, tag="qr")
                ta = ptmp.tile([P, 5, 32], F32, tag="ta")
                tb = ptmp.tile([P, 5, 32], F32, tag="tb")
                nc.vector.tensor_mul(ta, q1, cosb)
                nc.vector.tensor_mul(tb, q2, sinb)
                nc.vector.tensor_tensor(qr[:, 0:5, 0:32], ta, tb, ALU.subtract)
                tc2 = ptmp.tile([P, 5, 32], F32, tag="tc2")
                td = ptmp.tile([P, 5, 32], F32, tag="td")
                nc.vector.tensor_mul(tc2, q2, cosb)
                nc.vector.tensor_mul(td, q1, sinb)
                nc.vector.tensor_tensor(qr[:, 0:5, 32:64], tc2, td, ALU.add)
                nc.vector.tensor_copy(qr[:, 5, :], qr[:, 4, :])
                return qr

            def qk_transpose(m, qr):
                """DMA-transpose head pairs into [d, S] layout: q heads
                (2p, 2p+1) stack on partition halves of qT_sb[:, p]; k is
                duplicated so kT lands on BOTH halves (odd q heads must
                stream against a same-base-partition kT copy)."""
                for pair in range(2):
                    nc.sync.dma_start_transpose(
                        qT_sb[:, pair, ts(m, P)], qr[:, 2 * pair : 2 * pair + 2, :]
                    )
                nc.sync.dma_start_transpose(kT_sb[:, ts(m, P)], qr[:, 4:6, :])

            def scores_softmax(m):
                """scores + tanh-softcap + masked exp + row-normalize +
                DMA-transpose for chunk m; returns the pT tile."""
                km = (m + 1) * P
                p_m = ppool.tile([P, NQH, SEQ], F16, tag="p")
                rcs = []
                for h in range(NQH):
                    half = 64 * (h % 2)
                    lhsT = qT_sb[half : half + 64, h // 2, ts(m, P)]
                    t_h = tpool.tile([P, SEQ], F32, tag="t")
                    for base in range(0, km, 512):
                        w_sub = min(512, km - base)
                        pss = psum_s.tile([P, 512], F32, tag="s")
                        nc.tensor.matmul(
                            pss[:, 0:w_sub],
                            lhsT=lhsT,
                            rhs=kT_sb[half : half + 64, base : base + w_sub],
                            start=True,
                            stop=True,
                        )
                        nc.scalar.activation(
                            t_h[:, base : base + w_sub],
                            pss[:, 0:w_sub],
                            AF.Tanh,
                            scale=1.0 / (8.0 * CAP),
                        )
                    # causal mask on the diagonal chunk: set to -100 where
                    # strictly-upper, so exp(50*t) = 0 there and the
                    # accumulated denominator is correct.
                    nc.vector.copy_predicated(t_h[:, km - P : km], triu_sb, neg_sb)
                    den = small.tile([P, 1], F32, tag="den", name="den")
                    nc.scalar.activation(
                        p_m[:, h, 0:km], t_h[:, 0:km], AF.Exp, scale=CAP,
                        accum_out=den,
                    )
                    rc = small.tile([P, 1], F32, tag="rc", name="rc")
                    nc.vector.reciprocal(rc, den)
                    rcs.append(rc)
                # normalize p rows (per-partition scalar), then transpose
                for h in range(NQH):
                    nc.vector.tensor_scalar_mul(
                        p_m[:, h, 0:km], p_m[:, h, 0:km], rcs[h]
                    )
                pT = ptp.tile([P, MC, NQH, P], F16, tag="pT")
                for h in range(NQH):
                    nc.sync.dma_start_transpose(
                        pT[:, 0 : m + 1, h, :], p_m[:, h, 0:km]
                    )
                return pT

            def pv_y(m, pT):
                """PV + output projection + y store for chunk m."""
                # PV: v stationary, 2 head-pairs streamed into partition halves
                pv = psum_pv.tile([P, 2, P], F32, tag="pv", name="pv")
                for kc in range(m + 1):
                    nc.tensor.matmul(
                        pv[0:64, :, :],
                        lhsT=v_sb[:, kc, :],
                        rhs=pT[:, kc, 0:2, :],
                        start=(kc == 0),
                        stop=(kc == m),
                        skip_group_check=True,
                    )
                    nc.tensor.matmul(
                        pv[64:128, :, :],
                        lhsT=v_sb[:, kc, :],
                        rhs=pT[:, kc, 2:4, :],
                        start=(kc == 0),
                        stop=(kc == m),
                        skip_group_check=True,
                    )
                oT = opool.tile([P, 2, P], F16, tag="oT")
                nc.vector.tensor_copy(oT, pv)
                y_sb = ysb.tile([P, D_MODEL], F32, tag="ysb")
                for nh in range(2):
                    yp = psum_y.tile([P, 512], F32, tag="y", name="y")
                    for g in range(2):
                        nc.tensor.matmul(
                            yp,
                            lhsT=oT[:, g, :],
                            rhs=wo_sb[:, g, ts(nh, 512)],
                            start=(g == 0),
                            stop=(g == 1),
                        )
                    nc.vector.tensor_copy(y_sb[:, ts(nh, 512)], yp)
                nc.gpsimd.dma_start(y_d[ts(m, P), :], y_sb)

            # software pipeline: proj/rope/transpose of m+1 overlap attn of m
            pj = proj_front(0)
            qr = proj_back(0, pj)
            qk_transpose(0, qr)
            for m in range(MC):
                if m + 1 < MC:
                    pj = proj_front(m + 1)
                    qr = proj_back(m + 1, pj)
                pT = scores_softmax(m)
                pv_y(m, pT)
                if m + 1 < MC:
                    qk_transpose(m + 1, qr)
    nc.finalize()
    return nc


def get_nc():
    global _nc_cache
    if _nc_cache is None:
        _nc_cache = _build_nc()
    return _nc_cache


def make_in_maps(x, w_q, w_k, w_v, w_o):
    x = np.asarray(x, np.float32)
    w_q = np.asarray(w_q, np.float32)
    w_k = np.asarray(w_k, np.float32)
    w_v = np.asarray(w_v, np.float32)
    w_o = np.asarray(w_o, np.float32)

    inv_freq = 1.0 / (THETA ** (np.arange(0, HD, 2, dtype=np.float32) / HD))
    freqs = np.arange(SEQ, dtype=np.float32)[:, None] * inv_freq[None, :]
    cs = np.concatenate(
        [np.cos(freqs), np.sin(freqs)], axis=1
    ).astype(np.float32)  # (S, 64)
    triu = (1 - np.tril(np.ones((P, P), np.uint8))).astype(np.uint8)

    in_maps = []
    for c in range(N_CORES):
        b, g = divmod(c, 4)
        wqkv = np.concatenate(
            [
                w_q[:, g * 256 : (g + 1) * 256],
                w_k[:, g * 64 : (g + 1) * 64],
                w_v[:, g * 64 : (g + 1) * 64],
            ],
            axis=1,
        ).astype(np.float16)
        # wo rows permuted: y-contraction chunk 0 = heads (0,2), chunk 1 =
        # heads (1,3) (heads stacked on partition halves by the PV matmuls)
        wo_c = w_o[g * 256 : (g + 1) * 256, :].reshape(4, 64, D_MODEL)
        wo_p = wo_c[[0, 2, 1, 3]].reshape(256, D_MODEL).astype(np.float16)
        in_maps.append(
            {
                "xT": np.ascontiguousarray(x[b].T).astype(np.float16),
                "wqkv": np.ascontiguousarray(wqkv),
                "wo": np.ascontiguousarray(wo_p),
                "cs": cs,
                "triu": triu,
            }
        )
    return in_maps


def kernel(x, w_q, w_k, w_v, w_o):
    from concourse.bass_utils import run_bass_kernel_spmd

    nc = get_nc()
    in_maps = make_in_maps(x, w_q, w_k, w_v, w_o)
    res = run_bass_kernel_spmd(nc, in_maps, list(range(N_CORES))).results
    y = np.zeros((2, SEQ, D_MODEL), np.float32)
    for c in range(N_CORES):
        y[c // 4] += res[c]["y"]
    return y
